# revision 18
# baseline (speedup 1.0000x reference)
"""AttnDecoderRNN step on 8 Trainium2 NeuronCores.

Strategy:
  Phase A (batch-sharded, 16 batches/core): attention over enc_outputs +
    2-layer GRU cell -> new_hidden shard + h1^T shard.
    - enc_outputs is consumed ONLY in transposed bf16 layout encT [b, H, S]
      (host casts/transposes once); the attention-weighted sum Xa is a fused
      DVE tensor_tensor_reduce over the free (S) axis, so the natural layout
      is never needed on device.
    - scores for all 16 batches accumulate into one PSUM [16, S] via a
      V-in-column-b stationary trick, so softmax runs batched on partitions.
  Phase B (vocab-sharded, 4000 vocab/core): logits = h1 @ Wout^T + bout and
    log-softmax partials (row max / sumexp); host combines the 8 partial
    stats and applies the scalar per-row offset.
"""

import sys

import numpy as np
import ml_dtypes

for _p in ("/opt/trn_rl_repo",):
    if _p not in sys.path:
        sys.path.insert(0, _p)

B, S, H, E, V_SZ = 128, 2048, 256, 256, 32000
NC = 8
BP = B // NC          # batches per core
VP = V_SZ // NC       # vocab per core
SC = 512              # s-chunk for the u matmul / scores
NSC = S // SC         # 4 chunks
BF16 = ml_dtypes.bfloat16

_CACHE = {}


def _build_phase_a(stage="full"):
    import concourse.bacc as bacc
    import concourse.mybir as mybir
    from concourse.tile import TileContext

    dt = mybir.dt
    AF = mybir.ActivationFunctionType
    ALU = mybir.AluOpType

    nc = bacc.Bacc("TRN2", target_bir_lowering=False, debug=False,
                   enable_asserts=False, num_devices=NC)

    f32 = dt.float32
    bf = dt.bfloat16
    dbg = None
    if stage != "full":
        dbg = nc.dram_tensor("dbg", [128, S], f32, kind="ExternalOutput").ap()

    encT = nc.dram_tensor("encT", [BP, H, S], bf, kind="ExternalInput").ap()
    encN = nc.dram_tensor("encN", [BP, S, H], bf, kind="ExternalInput").ap()
    w1 = nc.dram_tensor("w1", [H, H], bf, kind="ExternalInput").ap()
    vv = nc.dram_tensor("vv", [2, 128, BP * BP], bf, kind="ExternalInput").ap()
    w2hT = nc.dram_tensor("w2hT", [H, BP], f32, kind="ExternalInput").ap()
    r1T = nc.dram_tensor("r1T", [H, BP], f32, kind="ExternalInput").ap()
    w3b = nc.dram_tensor("w3b", [H, H], f32, kind="ExternalInput").ap()
    wihT0 = nc.dram_tensor("wihT0", [H, 3 * H], f32, kind="ExternalInput").ap()
    whhT0 = nc.dram_tensor("whhT0", [H, 3 * H], f32, kind="ExternalInput").ap()
    wihT1 = nc.dram_tensor("wihT1", [H, 3 * H], f32, kind="ExternalInput").ap()
    whhT1 = nc.dram_tensor("whhT1", [H, 3 * H], f32, kind="ExternalInput").ap()
    bias = nc.dram_tensor("bias", [2, 4, H], f32, kind="ExternalInput").ap()
    h0pT = nc.dram_tensor("h0pT", [H, BP], f32, kind="ExternalInput").ap()
    h1pT = nc.dram_tensor("h1pT", [H, BP], f32, kind="ExternalInput").ap()

    nh_out = nc.dram_tensor("nh_out", [2, BP, H], f32, kind="ExternalOutput").ap()
    h1T_out = nc.dram_tensor("h1T_out", [H, BP], f32, kind="ExternalOutput").ap()

    with TileContext(nc) as tc:
        with (
            tc.tile_pool(name="const", bufs=1) as cpool,
            tc.tile_pool(name="enc", bufs=1) as epool,
            tc.tile_pool(name="work", bufs=6) as wpool,
            tc.tile_pool(name="abc", bufs=2) as apool,
            tc.tile_pool(name="encn", bufs=8) as npool,
            tc.tile_pool(name="dram", bufs=1, space="DRAM") as dpool,
            tc.tile_pool(name="ups", bufs=2, space="PSUM") as upool,
            tc.tile_pool(name="sps", bufs=2, space="PSUM") as spool,
            tc.tile_pool(name="gps", bufs=3, space="PSUM") as gpool,
            tc.tile_pool(name="xps", bufs=1, space="PSUM") as xpool,
        ):
            # ---- constants / params to SBUF ----
            w1_sb = cpool.tile([128, 2, H], bf, tag="w1")
            nc.sync.dma_start(out=w1_sb[:, 0, :], in_=w1[0:128, :])
            nc.sync.dma_start(out=w1_sb[:, 1, :], in_=w1[128:256, :])
            vv_sb = cpool.tile([128, 2, BP * BP], bf, tag="vv")
            nc.sync.dma_start(out=vv_sb[:], in_=vv.rearrange("k p m -> p k m"))
            w2hT_sb = cpool.tile([128, 2, BP], f32, tag="w2hT")
            nc.sync.dma_start(out=w2hT_sb[:], in_=w2hT.rearrange("(k p) b -> p k b", p=128))
            r1T_sb = cpool.tile([128, 2, BP], f32, tag="r1T")
            nc.sync.dma_start(out=r1T_sb[:], in_=r1T.rearrange("(k p) b -> p k b", p=128))
            w3b_sb = cpool.tile([128, 2, H], f32, tag="w3b")
            nc.sync.dma_start(out=w3b_sb[:], in_=w3b.rearrange("(k p) m -> p k m", p=128))
            gw_sb = {}
            for name, t in (("wihT0", wihT0), ("whhT0", whhT0),
                            ("wihT1", wihT1), ("whhT1", whhT1)):
                g = cpool.tile([128, 2, 3 * H], f32, tag=name)
                nc.sync.dma_start(out=g[:], in_=t.rearrange("(k p) m -> p k m", p=128))
                gw_sb[name] = g
            bias_sb = cpool.tile([128, 2, 4, 2], f32, tag="bias")
            nc.sync.dma_start(
                out=bias_sb[:], in_=bias.rearrange("l t (c p) -> p l t c", p=128))
            hp_sb = {}
            for name, t in (("h0pT", h0pT), ("h1pT", h1pT)):
                g = cpool.tile([128, 2, BP], f32, tag=name)
                nc.sync.dma_start(out=g[:], in_=t.rearrange("(k p) b -> p k b", p=128))
                hp_sb[name] = g

            # ---- load all encT tiles (resident) ----
            enc_sb = {}
            for b in range(BP):
                for k in range(2):
                    t = epool.tile([128, S], bf, tag=f"enc{b}_{k}")
                    nc.sync.dma_start(out=t[:], in_=encT[b, 128 * k:128 * (k + 1), :])
                    enc_sb[(b, k)] = t

            # ---- scores: for each s-chunk, accumulate all batches ----
            scores_sb = cpool.tile([BP, S], f32, tag="scores")
            uts = {}
            for c in range(NSC):
                sps = spool.tile([BP, SC], f32, tag="sc")
                for b in range(BP):
                    for m in range(2):
                        ups = upool.tile([128, SC], f32, tag="u")
                        for k in range(2):
                            nc.tensor.matmul(
                                ups[:], w1_sb[:, k, 128 * m:128 * (m + 1)],
                                enc_sb[(b, k)][:, SC * c:SC * (c + 1)],
                                start=(k == 0), stop=(k == 1))
                        ut = wpool.tile([128, SC], bf, tag="ut")
                        nc.scalar.activation(ut[:], ups[:], AF.Tanh,
                                             bias=w2hT_sb[:, m, b:b + 1], scale=1.0)
                        uts[m] = ut
                    for m in range(2):
                        nc.tensor.matmul(
                            sps[:], vv_sb[:, m, BP * b:BP * (b + 1)], uts[m][:],
                            start=(b == 0 and m == 0), stop=(b == BP - 1 and m == 1))
                nc.vector.tensor_copy(scores_sb[:, SC * c:SC * (c + 1)], sps[:])

            if stage == "scores":
                nc.sync.dma_start(out=dbg[0:BP, :], in_=scores_sb[:])
            if stage in ("xa", "full"):
                # ---- softmax over s (batched on partitions 0..15) ----
                negmax = cpool.tile([BP, 1], f32, tag="negmax")
                nc.vector.tensor_reduce(negmax[:], scores_sb[:], mybir.AxisListType.X,
                                        ALU.max, negate=True)
                a_un = cpool.tile([BP, S], bf, tag="a_un")
                sumexp = cpool.tile([BP, 1], f32, tag="sumexp")
                nc.scalar.activation(a_un[:], scores_sb[:], AF.Exp,
                                     bias=negmax[:], scale=1.0, accum_out=sumexp[:])
                rcp = cpool.tile([BP, 1], f32, tag="rcp")
                nc.vector.reciprocal(rcp[:], sumexp[:])
                a_bf = cpool.tile([BP, S], bf, tag="a_bf")
                nc.vector.tensor_scalar_mul(a_bf[:], a_un[:], rcp[:])

                # ---- Xa^T[h, b] on PE: aT column stationary, natural enc moving ----
                xaT_sb = cpool.tile([128, 2, BP], f32, tag="xaT")
                NS128 = S // 128
                a_dr = dpool.tile([BP, S], bf, tag="a_dr")
                x_dr = dpool.tile([BP, H], f32, tag="x_dr")
                for b in range(BP):
                    nc.sync.dma_start(out=a_dr[b:b + 1, :], in_=a_bf[b:b + 1, :])
                    aTb = apool.tile([128, NS128], bf, tag="aTb")
                    nc.sync.dma_start(
                        out=aTb[:], in_=a_dr[b].rearrange("(sc p) -> p sc", p=128))
                    xps = xpool.tile([1, H], f32, tag="x")
                    for sc in range(NS128):
                        encN_t = npool.tile([128, H], bf, tag="encN")
                        nc.sync.dma_start(out=encN_t[:],
                                          in_=encN[b, 128 * sc:128 * (sc + 1), :])
                        nc.tensor.matmul(xps[:], aTb[:, sc:sc + 1], encN_t[:],
                                         start=(sc == 0), stop=(sc == NS128 - 1))
                    xrow = apool.tile([1, H], f32, tag="xrow")
                    nc.vector.tensor_copy(xrow[:], xps[:])
                    nc.sync.dma_start(out=x_dr[b:b + 1, :], in_=xrow[:])
                for k in range(2):
                    nc.sync.dma_start(
                        out=xaT_sb[:, k, :],
                        in_=x_dr[:].rearrange("b (k p) -> p k b", p=128)[:, k, :])

            if stage == "xa":
                nc.sync.dma_start(out=dbg[:, 0:2 * BP],
                                  in_=xaT_sb[:].rearrange("p k b -> p (k b)"))
            if stage == "full":
                # ---- res^T = W3b^T @ Xa^T + r1T ----
                resT_sb = cpool.tile([128, 2, BP], f32, tag="resT")
                for m in range(2):
                    rps = gpool.tile([128, BP], f32, tag="g")
                    for k in range(2):
                        nc.tensor.matmul(rps[:], w3b_sb[:, k, 128 * m:128 * (m + 1)],
                                         xaT_sb[:, k, :], start=(k == 0), stop=(k == 1))
                    nc.vector.tensor_tensor(out=resT_sb[:, m, :], in0=rps[:],
                                            in1=r1T_sb[:, m, :], op=ALU.add)

                # ---- 2 GRU layers ----
                def gate_mm(pairs, g, m):
                    ps = gpool.tile([128, BP], f32, tag="g")
                    n_mm = 2 * len(pairs)
                    i = 0
                    for wname, xin in pairs:
                        for k in range(2):
                            nc.tensor.matmul(
                                ps[:], gw_sb[wname][:, k, 128 * (2 * g + m):128 * (2 * g + m + 1)],
                                xin[:, k, :], start=(i == 0), stop=(i == n_mm - 1))
                            i += 1
                    return ps

                def gru(l, xT, hpT, wih, whh):
                    houtT = cpool.tile([128, 2, BP], f32, tag=f"h{l}T")
                    for m in range(2):
                        t = gate_mm(((wih, xT), (whh, hpT)), 0, m)
                        r = wpool.tile([128, BP], f32, tag="gtmp")
                        nc.scalar.activation(r[:], t[:], AF.Sigmoid,
                                             bias=bias_sb[:, l, 0, m:m + 1], scale=1.0)
                        t2 = gate_mm(((wih, xT), (whh, hpT)), 1, m)
                        z = wpool.tile([128, BP], f32, tag="gtmp")
                        nc.scalar.activation(z[:], t2[:], AF.Sigmoid,
                                             bias=bias_sb[:, l, 1, m:m + 1], scale=1.0)
                        gi_n = gate_mm(((wih, xT),), 2, m)
                        gh_n = gate_mm(((whh, hpT),), 2, m)
                        hn = wpool.tile([128, BP], f32, tag="gtmp")
                        nc.vector.tensor_scalar_add(hn[:], gh_n[:],
                                                    bias_sb[:, l, 3, m:m + 1])
                        rhn = wpool.tile([128, BP], f32, tag="gtmp")
                        nc.vector.tensor_tensor(out=rhn[:], in0=r[:], in1=hn[:],
                                                op=ALU.mult)
                        s_ = wpool.tile([128, BP], f32, tag="gtmp")
                        nc.vector.tensor_tensor(out=s_[:], in0=gi_n[:], in1=rhn[:],
                                                op=ALU.add)
                        n = wpool.tile([128, BP], f32, tag="gtmp")
                        nc.scalar.activation(n[:], s_[:], AF.Tanh,
                                             bias=bias_sb[:, l, 2, m:m + 1], scale=1.0)
                        d = wpool.tile([128, BP], f32, tag="gtmp")
                        nc.vector.tensor_tensor(out=d[:], in0=hpT[:, m, :], in1=n[:],
                                                op=ALU.subtract)
                        zd = wpool.tile([128, BP], f32, tag="gtmp")
                        nc.vector.tensor_tensor(out=zd[:], in0=z[:], in1=d[:],
                                                op=ALU.mult)
                        nc.vector.tensor_tensor(out=houtT[:, m, :], in0=n[:], in1=zd[:],
                                                op=ALU.add)
                    return houtT

                h0T = gru(0, resT_sb, hp_sb["h0pT"], "wihT0", "whhT0")
                h1T = gru(1, h0T, hp_sb["h1pT"], "wihT1", "whhT1")

                # ---- outputs ----
                for l, hT in ((0, h0T), (1, h1T)):
                    for k in range(2):
                        nc.sync.dma_start(
                            out=nh_out[l].rearrange("b (k p) -> k p b", p=128)[k],
                            in_=hT[:, k, :])
                nc.sync.dma_start(out=h1T_out.rearrange("(k p) b -> p k b", p=128),
                                  in_=h1T[:])

    nc.compile()
    return nc


def _build_phase_b():
    import concourse.bacc as bacc
    import concourse.mybir as mybir
    from concourse.tile import TileContext

    dt = mybir.dt
    AF = mybir.ActivationFunctionType
    ALU = mybir.AluOpType
    f32 = dt.float32
    f32r = dt.float32r

    nc = bacc.Bacc("TRN2", target_bir_lowering=False, debug=False,
                   enable_asserts=False, num_devices=NC)

    h1a = nc.dram_tensor("h1a", [H + 1, B], f32, kind="ExternalInput").ap()
    wouta = nc.dram_tensor("wouta", [H + 1, VP], f32, kind="ExternalInput").ap()
    lg_out = nc.dram_tensor("lg_out", [B, VP], f32, kind="ExternalOutput").ap()
    negmx_out = nc.dram_tensor("negmx_out", [B, 1], f32, kind="ExternalOutput").ap()
    se_out = nc.dram_tensor("se_out", [B, 1], f32, kind="ExternalOutput").ap()

    VCH = 500  # 8 chunks of 500 (N>=256 keeps f32r at full rate)

    with TileContext(nc) as tc:
        with (
            tc.tile_pool(name="const", bufs=1) as cpool,
            tc.tile_pool(name="ps", bufs=4, space="PSUM") as ppool,
        ):
            h1_sb = cpool.tile([128, 2, B], f32, tag="h1")
            nc.sync.dma_start(out=h1_sb[:], in_=h1a[0:H].rearrange("(k p) b -> p k b", p=128))
            ones_sb = cpool.tile([1, B], f32, tag="ones")
            nc.sync.dma_start(out=ones_sb[:], in_=h1a[H:H + 1, :])
            w_sb = cpool.tile([128, 2, VP], f32, tag="w")
            nc.sync.dma_start(out=w_sb[:], in_=wouta[0:H].rearrange("(k p) v -> p k v", p=128))
            bo_sb = cpool.tile([1, VP], f32, tag="bo")
            nc.sync.dma_start(out=bo_sb[:], in_=wouta[H:H + 1, :])

            lg_sb = cpool.tile([B, VP], f32, tag="lg")
            for c in range(VP // VCH):
                ps = ppool.tile([B, VCH], f32, tag="ps")
                sl = slice(VCH * c, VCH * (c + 1))
                for k in range(2):
                    nc.tensor.matmul(ps[:], h1_sb[:, k, :],
                                     w_sb[:, k, sl],
                                     start=(k == 0), stop=False)
                nc.tensor.matmul(ps[:], ones_sb[:],
                                 bo_sb[:, sl], start=False, stop=True)
                nc.scalar.activation(lg_sb[:, sl], ps[:], AF.Copy)

            negmx = cpool.tile([B, 1], f32, tag="negmx")
            nc.vector.tensor_reduce(negmx[:], lg_sb[:], mybir.AxisListType.X,
                                    ALU.max, negate=True)
            se = cpool.tile([B, 1], f32, tag="se")
            etrash = cpool.tile([B, VP], dt.bfloat16, tag="etrash")
            nc.scalar.activation(etrash[:], lg_sb[:], AF.Exp,
                                 bias=negmx[:], scale=1.0, accum_out=se[:])

            nc.sync.dma_start(out=lg_out[:], in_=lg_sb[:])
            nc.sync.dma_start(out=negmx_out[:], in_=negmx[:])
            nc.sync.dma_start(out=se_out[:], in_=se[:])

    nc.compile()
    return nc


def _get_programs():
    if "a" not in _CACHE:
        _CACHE["a"] = _build_phase_a()
        _CACHE["b"] = _build_phase_b()
    return _CACHE["a"], _CACHE["b"]


def kernel(inp, hidden, enc_outputs, emb, W1, W2, W3, b2, b3, V,
           Wih0, Whh0, bih0, bhh0, Wih1, Whh1, bih1, bhh1, Wout, bout,
           _trace=False, _timers=None):
    from concourse.bass_utils import run_bass_kernel_spmd

    t = {}
    import time as _time
    t0 = _time.perf_counter()

    inp = np.asarray(inp)
    hidden = np.asarray(hidden, dtype=np.float32)
    enc_outputs = np.asarray(enc_outputs, dtype=np.float32)

    # ---- host prep ----
    emb_inp = np.asarray(emb, dtype=np.float32)[np.asarray(inp)]          # [B, E]
    r1 = emb_inp @ np.asarray(W3, np.float32)[:E] + np.asarray(b3, np.float32)
    w2h = hidden[-1] @ np.asarray(W2, np.float32) + np.asarray(b2, np.float32)

    encT = np.ascontiguousarray(enc_outputs.transpose(0, 2, 1)).astype(BF16)
    encN_bf = enc_outputs.astype(BF16)

    w1_bf = np.asarray(W1, np.float32).astype(BF16)
    Vf = np.asarray(V, np.float32)
    vv = np.zeros((2, 128, BP, BP), np.float32)
    for k in range(2):
        for p in range(128):
            vv[k, p, range(BP), range(BP)] = Vf[128 * k + p]
    vv = vv.reshape(2, 128, BP * BP).astype(BF16)

    w3b = np.ascontiguousarray(np.asarray(W3, np.float32)[E:])
    wihT0 = np.ascontiguousarray(np.asarray(Wih0, np.float32).T)
    whhT0 = np.ascontiguousarray(np.asarray(Whh0, np.float32).T)
    wihT1 = np.ascontiguousarray(np.asarray(Wih1, np.float32).T)
    whhT1 = np.ascontiguousarray(np.asarray(Whh1, np.float32).T)
    bias = np.zeros((2, 4, H), np.float32)
    for l, (bi, bh) in enumerate(((bih0, bhh0), (bih1, bhh1))):
        bi = np.asarray(bi, np.float32); bh = np.asarray(bh, np.float32)
        bias[l, 0] = bi[0:H] + bh[0:H]
        bias[l, 1] = bi[H:2 * H] + bh[H:2 * H]
        bias[l, 2] = bi[2 * H:3 * H]
        bias[l, 3] = bh[2 * H:3 * H]

    t["prep"] = _time.perf_counter() - t0; t0 = _time.perf_counter()

    nca, ncb = _get_programs()
    t["build"] = _time.perf_counter() - t0; t0 = _time.perf_counter()

    in_maps_a = []
    for c in range(NC):
        bs = slice(BP * c, BP * (c + 1))
        in_maps_a.append(dict(
            encT=np.ascontiguousarray(encT[bs]),
            encN=np.ascontiguousarray(encN_bf[bs]),
            w1=w1_bf, vv=vv,
            w2hT=np.ascontiguousarray(w2h[bs].T),
            r1T=np.ascontiguousarray(r1[bs].T),
            w3b=w3b, wihT0=wihT0, whhT0=whhT0, wihT1=wihT1, whhT1=whhT1,
            bias=bias,
            h0pT=np.ascontiguousarray(hidden[0][bs].T),
            h1pT=np.ascontiguousarray(hidden[1][bs].T),
        ))
    resa = run_bass_kernel_spmd(nca, in_maps_a, core_ids=list(range(NC)),
                                trace=_trace)
    t["runA"] = _time.perf_counter() - t0; t0 = _time.perf_counter()

    new_hidden = np.concatenate([resa.results[c]["nh_out"] for c in range(NC)], axis=1)
    h1T_full = np.concatenate([resa.results[c]["h1T_out"] for c in range(NC)], axis=1)
    h1a = np.concatenate([h1T_full, np.ones((1, B), np.float32)], axis=0)
    h1a = np.ascontiguousarray(h1a)

    woutT = np.ascontiguousarray(
        np.concatenate([np.asarray(Wout, np.float32).T,
                        np.asarray(bout, np.float32)[None, :]], axis=0))
    in_maps_b = []
    for c in range(NC):
        vs = slice(VP * c, VP * (c + 1))
        in_maps_b.append(dict(h1a=h1a, wouta=np.ascontiguousarray(woutT[:, vs])))
    resb = run_bass_kernel_spmd(ncb, in_maps_b, core_ids=list(range(NC)),
                                trace=_trace)
    t["runB"] = _time.perf_counter() - t0; t0 = _time.perf_counter()

    mx = np.stack([-resb.results[c]["negmx_out"][:, 0] for c in range(NC)])  # [NC, B]
    se = np.stack([resb.results[c]["se_out"][:, 0] for c in range(NC)])      # [NC, B]
    gmax = mx.max(axis=0)                                                    # [B]
    gsum = (se * np.exp(mx - gmax[None, :])).sum(axis=0)                     # [B]
    offset = (gmax + np.log(gsum)).astype(np.float32)                        # [B]
    out = np.concatenate([resb.results[c]["lg_out"] for c in range(NC)], axis=1)
    out -= offset[:, None]
    t["post"] = _time.perf_counter() - t0

    if _timers is not None:
        _timers.update(t)
    if _trace:
        _CACHE["last_results"] = (resa, resb)
    return out, new_hidden
